# revision 8
# baseline (speedup 1.0000x reference)
"""Trainium2 Bass kernel for nn_Corener_57062935495475 (segment_reduce).

Contract: kernel(**inputs) takes FULL unsharded numpy inputs (as produced by
setup_inputs()) and returns the FULL output tuple
(entity_clf, rel_clf, mention_clf, ref_clf), matching the jax reference.

Sharding (8 cores, single SPMD program; every per-core difference rides in
the input DATA, never in the instruction stream):
  core c -> batch b = c // 4, quarter q = c % 4.
  Each core computes the masked-max pools and classifier MLPs for its quarter
  of entities/mentions/relations/references of its batch.  Relation pair
  features are produced as their own pools (mask rows replicated host-side
  per relation slot), so no cross-core communication and no gathers needed.

Device algorithm per core:
  - token embeddings in SBUF transposed: 6 tiles [128 H-partitions, 512 S].
  - masked max pool per span: PE rank-1 matmul broadcasts the additive mask
    row (0 / -1e30) across 128 partitions into PSUM, ACT copies it to SBUF,
    then one DVE tensor_tensor_reduce (add + max-accum) per (span, H-tile)
    writes the pooled column directly in matmul-rhs layout.
  - classifier MLPs: f32 PE matmuls (weights streamed from DRAM as 128x128
    lhsT tiles), relu+bias fused on ACT reading PSUM.  The shared CLS-context
    term of the span classifiers is computed on device once per core as
    v = Wctx @ ctx and folded into the ACT bias.
"""

import numpy as np

import concourse.bacc as bacc
import concourse.tile as tile
from concourse import mybir
from concourse.bass_utils import run_bass_kernel_spmd

F32 = mybir.dt.float32
AF = mybir.ActivationFunctionType
ALU = mybir.AluOpType

# Problem shapes (hardcoded; kernel.py must be self-contained).
B, S, H = 2, 512, 768
E, M, R, R2 = 64, 64, 128, 128
NER_C, REL_C, SIZE_E, CLS = 10, 8, 32, 101
NEG = -1e30
HT = H // 128          # 6 H-tiles
N_CORES = 8
GROUP = 4              # cores per batch
EQ, MQ, RQ, R2Q = E // GROUP, M // GROUP, R // GROUP, R2 // GROUP  # 16,16,32,32
# negmask row layout (per core):
#   [ent 16 | men 16 | relctx 32 | refctx 32 | rpair0 32 | rpair1 32
#    | cpair0 32 | cpair1 32]  = 224 rows, padded to 256 = [128, 2, S]
ROW_ENT, ROW_MEN = 0, EQ
ROW_RCTX, ROW_CCTX = EQ + MQ, EQ + MQ + RQ
ROW_RP0 = EQ + MQ + RQ + R2Q
ROW_RP1 = ROW_RP0 + RQ
ROW_CP0 = ROW_RP1 + RQ
ROW_CP1 = ROW_CP0 + R2Q
NPOOL = ROW_CP1 + R2Q  # 224
NM_SLOTS = 3           # negmask free slots: rows at [32*((i//32)%3) + i%32, i//96]
MAXNEG_INIT = -3.0e38


def _tiles_T(w, k_pad):
    """w: [out, in] (torch convention).  Returns lhsT tiles
    [nct, npg, 128, 128] with tile[ct, pg][k, m] = w[128*pg+m, 128*ct+k],
    zero-padded on the contraction (in) dim to k_pad."""
    out_d, in_d = w.shape
    assert out_d % 128 == 0
    wT = np.zeros((k_pad, out_d), np.float32)
    wT[:in_d, :] = np.ascontiguousarray(w.T, dtype=np.float32)
    nct, npg = k_pad // 128, out_d // 128
    t = wT.reshape(nct, 128, npg, 128).transpose(0, 2, 1, 3)
    return np.ascontiguousarray(t, dtype=np.float32)


def _head_T(w, c_dim):
    """w: [C, 768] -> [6, 128, C] lhsT tiles."""
    wT = np.ascontiguousarray(w.T, dtype=np.float32)  # [768, C]
    return np.ascontiguousarray(wT.reshape(HT, 128, c_dim), dtype=np.float32)


def build_program():
    """Builds the SPMD Bass program (identical on all cores)."""
    nc = bacc.Bacc("TRN2", target_bir_lowering=False, debug=False,
                   num_devices=N_CORES)

    dram = {}

    def din(name, shape):
        dram[name] = nc.dram_tensor(name, list(shape), F32, kind="ExternalInput")

    def dout(name, shape):
        dram[name] = nc.dram_tensor(name, list(shape), F32, kind="ExternalOutput")

    din("embT", (128, HT, S))        # embT[p, j, s] = emb[b, s, 128j+p]
    din("negmask", (96, NM_SLOTS, S))  # row i -> [32*((i//32)%3) + i%32, i//96]
    din("indsel", (96, 32, 128))     # indsel[32q+c, k, p] = (c == k)
    din("identity", (128, 128))
    din("ctx_col", (128, HT))        # ctx_col[p, j] = emb[b, cls_s, 128j+p]
    din("ner_sz_static", (128, EQ))  # rows 0:32 sz^T, rest zero
    din("emd_sz_static", (128, MQ))
    din("rel_sz_static", (128, RQ))  # rows 0:64 szpair^T, rest zero
    din("cr_sz_static", (128, R2Q))
    for n in ["ner_b1", "emd_b1", "rel_b1", "cr_b1", "rel_b2", "cr_b2"]:
        din(n, (128, HT))
    din("head_biases", (128, 4))     # cols: ner(10), emd(2), rel(8), cr(1)
    din("ner_wctxT", (HT, HT, 128, 128))
    din("emd_wctxT", (HT, HT, 128, 128))
    din("ner_w1T", (7, HT, 128, 128))
    din("emd_w1T", (7, HT, 128, 128))
    din("rel_w1T", (19, HT, 128, 128))
    din("cr_w1T", (19, HT, 128, 128))
    din("rel_w2T", (HT, HT, 128, 128))
    din("cr_w2T", (HT, HT, 128, 128))
    din("ner_headT", (HT, 128, NER_C))
    din("emd_headT", (HT, 128, 2))
    din("rel_headT", (HT, 128, REL_C))
    din("cr_headT", (HT, 128, 1))

    dout("ner_out", (NER_C, EQ))
    dout("emd_out", (2, MQ))
    dout("rel_out", (REL_C, RQ))
    dout("cr_out", (1, R2Q))

    with tile.TileContext(nc) as tc:
        _kernel_body(tc, dram)

    nc.compile()
    return nc


def _kernel_body(tc, dram):
    nc = tc.nc
    from contextlib import ExitStack
    ctx = ExitStack()
    with ctx:
        const_pool = ctx.enter_context(tc.tile_pool(name="const", bufs=1))
        pools_pool = ctx.enter_context(tc.tile_pool(name="pools", bufs=1))
        mpsum_pool = ctx.enter_context(tc.tile_pool(name="mpsum", bufs=3, space="PSUM"))
        w_pool = ctx.enter_context(tc.tile_pool(name="wts", bufs=96))
        mm_psum = ctx.enter_context(tc.tile_pool(name="mmpsum", bufs=2, space="PSUM"))
        act_pool = ctx.enter_context(tc.tile_pool(name="acts", bufs=1))
        misc_pool = ctx.enter_context(tc.tile_pool(name="misc", bufs=1))

        def load(name, shape, pool=const_pool):
            t = pool.tile(list(shape), F32, name=name, tag=name)
            nc.sync.dma_start(out=t[...], in_=dram[name][...])
            return t

        embT = load("embT", (128, HT, S))
        negmask = load("negmask", (96, NM_SLOTS, S))
        indsel = load("indsel", (96, 32, 128))
        identity = load("identity", (128, 128))
        ctx_col = load("ctx_col", (128, HT))
        statics = {n: load(n, (128, q)) for n, q in
                   [("ner_sz_static", EQ), ("emd_sz_static", MQ),
                    ("rel_sz_static", RQ), ("cr_sz_static", R2Q)]}
        biases = {n: load(n, (128, HT)) for n in
                  ["ner_b1", "emd_b1", "rel_b1", "cr_b1", "rel_b2", "cr_b2"]}
        head_biases = load("head_biases", (128, 4))

        def wtile(name, idx, shape):
            t = w_pool.tile(list(shape), F32, name="w", tag="w")
            nc.sync.dma_start(out=t[...], in_=dram[name][idx])
            return t

        # ---- ctx projection: v = Wctx^T.T @ ctx; bias_vec = v + b1 ----
        bias_vec = {}
        for path, wname, bname in [("ner", "ner_wctxT", "ner_b1"),
                                   ("emd", "emd_wctxT", "emd_b1")]:
            bv = misc_pool.tile([128, HT], F32, name=f"bv_{path}", tag=f"bv_{path}")
            for pg in range(HT):
                ps = mm_psum.tile([128, 1], F32, name="vps", tag="mmps")
                for ct in range(HT):
                    w = wtile(wname, (ct, pg), (128, 128))
                    nc.tensor.matmul(ps[...], w[...], ctx_col[:, ct:ct + 1],
                                     start=(ct == 0), stop=(ct == HT - 1))
                nc.vector.tensor_add(bv[:, pg:pg + 1], ps[...],
                                     biases[bname][:, pg:pg + 1])
            bias_vec[path] = bv

        # ---- masked max pools ----
        ptiles = {
            "ent": pools_pool.tile([128, HT, EQ], F32, name="entp", tag="entp"),
            "men": pools_pool.tile([128, HT, MQ], F32, name="menp", tag="menp"),
            "rctx": pools_pool.tile([128, HT, RQ], F32, name="rctxp", tag="rctxp"),
            "cctx": pools_pool.tile([128, HT, R2Q], F32, name="cctxp", tag="cctxp"),
            "rp0": pools_pool.tile([128, HT, RQ], F32, name="rp0", tag="rp0"),
            "rp1": pools_pool.tile([128, HT, RQ], F32, name="rp1", tag="rp1"),
            "cp0": pools_pool.tile([128, HT, R2Q], F32, name="cp0", tag="cp0"),
            "cp1": pools_pool.tile([128, HT, R2Q], F32, name="cp1", tag="cp1"),
        }
        layout = [("ent", ROW_ENT, EQ), ("men", ROW_MEN, MQ),
                  ("rctx", ROW_RCTX, RQ), ("cctx", ROW_CCTX, R2Q),
                  ("rp0", ROW_RP0, RQ), ("rp1", ROW_RP1, RQ),
                  ("cp0", ROW_CP0, R2Q), ("cp1", ROW_CP1, R2Q)]

        NB = 2  # spans per fused psum/reduce block
        for key, row0, n in layout:
            ptile = ptiles[key]
            for blk in range(0, n, NB):
                for j in range(HT):
                    ps = mpsum_pool.tile([128, NB, S], F32, name="maskps",
                                         tag="maskps")
                    for u in range(NB):
                        row = row0 + blk + u
                        qd, k, slot = (row // 32) % 3, row % 32, row // 96
                        nc.tensor.matmul(ps[:, u, :], identity[...],
                                         embT[:, j, :], start=True, stop=False)
                        nc.tensor.matmul(ps[:, u, :],
                                         indsel[32 * qd:32 * qd + 32, k, :],
                                         negmask[32 * qd:32 * qd + 32, slot, :],
                                         start=False, stop=True)
                    nc.vector.tensor_reduce(ptile[:, j, blk:blk + NB], ps[...],
                                            axis=mybir.AxisListType.X,
                                            op=ALU.max)

        # ---- classifier MLPs ----
        def l1_span(path, pool_t, nspans):
            h1 = act_pool.tile([128, HT, nspans], F32, name=f"h1_{path}", tag=f"h1_{path}")
            for pg in range(HT):
                ps = mm_psum.tile([128, nspans], F32, name="l1ps", tag="mmps")
                for ct in range(7):
                    w = wtile(f"{path}_w1T", (ct, pg), (128, 128))
                    rhs = (pool_t[:, ct, :] if ct < HT
                           else statics[f"{path}_sz_static"][...])
                    nc.tensor.matmul(ps[...], w[...], rhs,
                                     start=(ct == 0), stop=(ct == 6))
                nc.scalar.activation(h1[:, pg, :], ps[...], AF.Relu,
                                     bias=bias_vec[path][:, pg:pg + 1])
            return h1

        def l1_rel(path, ctx_t, p0, p1, nspans):
            h1 = act_pool.tile([128, HT, nspans], F32, name=f"h1_{path}", tag=f"h1_{path}")
            for pg in range(HT):
                ps = mm_psum.tile([128, nspans], F32, name="l1ps", tag="mmps")
                for ct in range(19):
                    w = wtile(f"{path}_w1T", (ct, pg), (128, 128))
                    if ct < 6:
                        rhs = ctx_t[:, ct, :]
                    elif ct < 12:
                        rhs = p0[:, ct - 6, :]
                    elif ct < 18:
                        rhs = p1[:, ct - 12, :]
                    else:
                        rhs = statics[f"{path}_sz_static"][...]
                    nc.tensor.matmul(ps[...], w[...], rhs,
                                     start=(ct == 0), stop=(ct == 18))
                nc.scalar.activation(h1[:, pg, :], ps[...], AF.Relu,
                                     bias=biases[f"{path}_b1"][:, pg:pg + 1])
            return h1

        def l2(path, h1, nspans):
            h2 = act_pool.tile([128, HT, nspans], F32, name=f"h2_{path}", tag=f"h2_{path}")
            for pg in range(HT):
                ps = mm_psum.tile([128, nspans], F32, name="l2ps", tag="mmps")
                for ct in range(HT):
                    w = wtile(f"{path}_w2T", (ct, pg), (128, 128))
                    nc.tensor.matmul(ps[...], w[...], h1[:, ct, :],
                                     start=(ct == 0), stop=(ct == HT - 1))
                nc.scalar.activation(h2[:, pg, :], ps[...], AF.Relu,
                                     bias=biases[f"{path}_b2"][:, pg:pg + 1])
            return h2

        def head(path, hin, nspans, c_dim, bias_col, out_name):
            ps = mm_psum.tile([c_dim, nspans], F32, name="headps", tag="mmps")
            for ct in range(HT):
                w = wtile(f"{path}_headT", (ct,), (128, c_dim))
                nc.tensor.matmul(ps[...], w[...], hin[:, ct, :],
                                 start=(ct == 0), stop=(ct == HT - 1))
            out_sb = misc_pool.tile([c_dim, nspans], F32, name=f"out_{path}", tag=f"out_{path}")
            nc.scalar.activation(out_sb[...], ps[...], AF.Identity,
                                 bias=head_biases[0:c_dim, bias_col:bias_col + 1])
            nc.sync.dma_start(out=dram[out_name][...], in_=out_sb[...])

        head("ner", l1_span("ner", ptiles["ent"], EQ), EQ, NER_C, 0, "ner_out")
        head("emd", l1_span("emd", ptiles["men"], MQ), MQ, 2, 1, "emd_out")
        rel_h2 = l2("rel", l1_rel("rel", ptiles["rctx"], ptiles["rp0"],
                                  ptiles["rp1"], RQ), RQ)
        head("rel", rel_h2, RQ, REL_C, 2, "rel_out")
        cr_h2 = l2("cr", l1_rel("cr", ptiles["cctx"], ptiles["cp0"],
                                ptiles["cp1"], R2Q), R2Q)
        head("cr", cr_h2, R2Q, 1, 3, "cr_out")


def _pack_core_inputs(b, q, host):
    """Per-core input map (all f32, pre-packed partition-major)."""
    im = {}
    im["embT"] = host["embT"][b]
    im["indsel"] = host["indsel"]
    im["identity"] = np.eye(128, dtype=np.float32)
    im["ctx_col"] = host["ctx_col"][b]

    sl = lambda a, n: a[b][q * n:(q + 1) * n]          # quarter rows [n, S]
    rels, refs = host["rels"][b], host["refs"][b]      # [R, 2] ints
    myrels = rels[q * RQ:(q + 1) * RQ]
    myrefs = refs[q * R2Q:(q + 1) * R2Q]
    rows = np.concatenate([
        sl(host["neg_ent"], EQ),
        sl(host["neg_men"], MQ),
        sl(host["neg_rel"], RQ),
        sl(host["neg_ref"], R2Q),
        host["neg_ent"][b][myrels[:, 0]],
        host["neg_ent"][b][myrels[:, 1]],
        host["neg_men"][b][myrefs[:, 0]],
        host["neg_men"][b][myrefs[:, 1]],
    ], axis=0)
    nm = np.zeros((96, NM_SLOTS, S), np.float32)
    for i in range(NPOOL):
        nm[32 * ((i // 32) % 3) + i % 32, i // 96] = rows[i]
    im["negmask"] = np.ascontiguousarray(nm)

    def sz_static(sz_rows, n):
        t = np.zeros((128, n), np.float32)
        t[:sz_rows.shape[1], :] = sz_rows.T
        return t

    im["ner_sz_static"] = sz_static(sl(host["sz_ent"], EQ), EQ)
    im["emd_sz_static"] = sz_static(sl(host["sz_men"], MQ), MQ)
    im["rel_sz_static"] = sz_static(sl(host["szp_rel"], RQ), RQ)
    im["cr_sz_static"] = sz_static(sl(host["szp_ref"], R2Q), R2Q)

    for n in ["ner_b1", "emd_b1", "rel_b1", "cr_b1", "rel_b2", "cr_b2",
              "head_biases", "ner_wctxT", "emd_wctxT", "ner_w1T", "emd_w1T",
              "rel_w1T", "cr_w1T", "rel_w2T", "cr_w2T", "ner_headT",
              "emd_headT", "rel_headT", "cr_headT"]:
        im[n] = host[n]
    return im


_PROGRAM_CACHE = []


def kernel(**inputs):
    inputs = {k: np.asarray(v) for k, v in inputs.items()}
    emb = inputs["token_embedding"].astype(np.float32)

    def negm(mask):
        return np.where(mask != 0, 0.0, NEG).astype(np.float32)

    ind = np.zeros((96, 32, 128), np.float32)
    for qd in range(3):
        for k in range(32):
            ind[32 * qd + k, k, :] = 1.0
    host = {
        "indsel": ind,
        "rels": np.asarray(inputs["relations"]),
        "refs": np.asarray(inputs["references"]),
        "neg_ent": negm(inputs["entity_masks"]),
        "neg_men": negm(inputs["mention_masks"]),
        "neg_rel": negm(inputs["rel_masks"]),
        "neg_ref": negm(inputs["references_masks"]),
        "sz_ent": inputs["ner_size_emb"][inputs["entity_sizes"]].astype(np.float32),
        "sz_men": inputs["emd_size_emb"][inputs["mention_sizes"]].astype(np.float32),
    }
    host["szp_rel"] = host["sz_ent"][
        np.arange(B)[:, None, None], host["rels"]].reshape(B, R, 2 * SIZE_E)
    host["szp_ref"] = host["sz_men"][
        np.arange(B)[:, None, None], host["refs"]].reshape(B, R2, 2 * SIZE_E)

    host["embT"] = np.ascontiguousarray(
        emb.transpose(0, 2, 1).reshape(B, HT, 128, S).transpose(0, 2, 1, 3),
        dtype=np.float32)
    ctx_cols = []
    for b in range(B):
        cls_s = int(np.argmax(np.asarray(inputs["input_ids"][b]) == CLS))
        ctx_cols.append(np.ascontiguousarray(
            emb[b, cls_s].reshape(HT, 128).T, dtype=np.float32))
    host["ctx_col"] = np.stack(ctx_cols)

    ner_rep_w = inputs["ner_rep_w"].astype(np.float32)
    emd_rep_w = inputs["emd_rep_w"].astype(np.float32)
    host["ner_wctxT"] = _tiles_T(ner_rep_w[:, :H], H)
    host["emd_wctxT"] = _tiles_T(emd_rep_w[:, :H], H)
    host["ner_w1T"] = _tiles_T(ner_rep_w[:, H:], 7 * 128)
    host["emd_w1T"] = _tiles_T(emd_rep_w[:, H:], 7 * 128)
    host["rel_w1T"] = _tiles_T(inputs["rel_w1"].astype(np.float32), 19 * 128)
    host["cr_w1T"] = _tiles_T(inputs["cr_w1"].astype(np.float32), 19 * 128)
    host["rel_w2T"] = _tiles_T(inputs["rel_w2"].astype(np.float32), H)
    host["cr_w2T"] = _tiles_T(inputs["cr_w2"].astype(np.float32), H)
    host["ner_headT"] = _head_T(inputs["ner_head_w"].astype(np.float32), NER_C)
    host["emd_headT"] = _head_T(inputs["emd_head_w"].astype(np.float32), 2)
    host["rel_headT"] = _head_T(inputs["rel_w3"].astype(np.float32), REL_C)
    host["cr_headT"] = _head_T(inputs["cr_w3"].astype(np.float32), 1)

    def pgmaj(v):
        return np.ascontiguousarray(
            np.asarray(v, np.float32).reshape(HT, 128).T, dtype=np.float32)

    for n, src in [("ner_b1", "ner_rep_b"), ("emd_b1", "emd_rep_b"),
                   ("rel_b1", "rel_b1"), ("cr_b1", "cr_b1"),
                   ("rel_b2", "rel_b2"), ("cr_b2", "cr_b2")]:
        host[n] = pgmaj(inputs[src])
    hb = np.zeros((128, 4), np.float32)
    hb[:NER_C, 0] = inputs["ner_head_b"]
    hb[:2, 1] = inputs["emd_head_b"]
    hb[:REL_C, 2] = inputs["rel_b3"]
    hb[:1, 3] = inputs["cr_b3"]
    host["head_biases"] = hb

    in_maps = [_pack_core_inputs(c // GROUP, c % GROUP, host)
               for c in range(N_CORES)]

    if not _PROGRAM_CACHE:
        _PROGRAM_CACHE.append(build_program())
    nc = _PROGRAM_CACHE[0]

    res = run_bass_kernel_spmd(nc, in_maps, list(range(N_CORES)))
    return assemble_outputs(res.results)


def assemble_outputs(results):
    entity_clf = np.zeros((B, E, NER_C), np.float32)
    mention_clf = np.zeros((B, M, 2), np.float32)
    rel_clf = np.zeros((B, R, REL_C), np.float32)
    ref_clf = np.zeros((B, R2, 1), np.float32)
    for c in range(N_CORES):
        b, q = c // GROUP, c % GROUP
        entity_clf[b, q * EQ:(q + 1) * EQ] = results[c]["ner_out"].T
        mention_clf[b, q * MQ:(q + 1) * MQ] = results[c]["emd_out"].T
        rel_clf[b, q * RQ:(q + 1) * RQ] = results[c]["rel_out"].T
        ref_clf[b, q * R2Q:(q + 1) * R2Q] = results[c]["cr_out"].T
    return entity_clf, rel_clf, mention_clf, ref_clf


# revision 10
# speedup vs baseline: 1.3877x; 1.3877x over previous
"""Trainium2 Bass kernel for nn_Corener_57062935495475 (segment_reduce).

Contract: kernel(**inputs) takes FULL unsharded numpy inputs (as produced by
setup_inputs()) and returns the FULL output tuple
(entity_clf, rel_clf, mention_clf, ref_clf), matching the jax reference.

Sharding (8 cores, single SPMD program; every per-core difference rides in
the input DATA, never in the instruction stream):
  core c -> batch b = c // 4, quarter q = c % 4.
  Each core computes the masked-max pools and classifier MLPs for its quarter
  of entities/mentions/relations/references of its batch.  Relation pair
  features are produced as their own pools (mask rows replicated host-side
  per relation slot), so no cross-core communication and no gathers needed.

Device algorithm per core:
  - token embeddings in SBUF transposed: 6 tiles [128 H-partitions, 512 S].
  - masked max pool per span: PE rank-1 matmul broadcasts the additive mask
    row (0 / -1e30) across 128 partitions into PSUM, ACT copies it to SBUF,
    then one DVE tensor_tensor_reduce (add + max-accum) per (span, H-tile)
    writes the pooled column directly in matmul-rhs layout.
  - classifier MLPs: f32 PE matmuls (weights streamed from DRAM as 128x128
    lhsT tiles), relu+bias fused on ACT reading PSUM.  The shared CLS-context
    term of the span classifiers is computed on device once per core as
    v = Wctx @ ctx and folded into the ACT bias.
"""

import numpy as np

import concourse.bacc as bacc
import concourse.tile as tile
from concourse import mybir
from concourse.bass_utils import run_bass_kernel_spmd

F32 = mybir.dt.float32
AF = mybir.ActivationFunctionType
ALU = mybir.AluOpType

# Problem shapes (hardcoded; kernel.py must be self-contained).
B, S, H = 2, 512, 768
E, M, R, R2 = 64, 64, 128, 128
NER_C, REL_C, SIZE_E, CLS = 10, 8, 32, 101
NEG = -1e30
HT = H // 128          # 6 H-tiles
N_CORES = 8
GROUP = 4              # cores per batch
EQ, MQ, RQ, R2Q = E // GROUP, M // GROUP, R // GROUP, R2 // GROUP  # 16,16,32,32
# negmask row layout (per core):
#   [ent 16 | men 16 | relctx 32 | refctx 32 | rpair0 32 | rpair1 32
#    | cpair0 32 | cpair1 32]  = 224 rows, padded to 256 = [128, 2, S]
ROW_ENT, ROW_MEN = 0, EQ
ROW_RCTX, ROW_CCTX = EQ + MQ, EQ + MQ + RQ
ROW_RP0 = EQ + MQ + RQ + R2Q
ROW_RP1 = ROW_RP0 + RQ
ROW_CP0 = ROW_RP1 + RQ
ROW_CP1 = ROW_CP0 + R2Q
NPOOL = ROW_CP1 + R2Q  # 224
NM_SLOTS = 3           # negmask free slots: rows at [32*((i//32)%3) + i%32, i//96]
MAXNEG_INIT = -3.0e38


def _tiles_T(w, k_pad):
    """w: [out, in] (torch convention).  Returns lhsT tiles
    [nct, npg, 128, 128] with tile[ct, pg][k, m] = w[128*pg+m, 128*ct+k],
    zero-padded on the contraction (in) dim to k_pad."""
    out_d, in_d = w.shape
    assert out_d % 128 == 0
    wT = np.zeros((k_pad, out_d), np.float32)
    wT[:in_d, :] = np.ascontiguousarray(w.T, dtype=np.float32)
    nct, npg = k_pad // 128, out_d // 128
    t = wT.reshape(nct, 128, npg, 128).transpose(0, 2, 1, 3)
    return np.ascontiguousarray(t, dtype=np.float32)


def _head_T(w, c_dim):
    """w: [C, 768] -> [6, 128, C] lhsT tiles."""
    wT = np.ascontiguousarray(w.T, dtype=np.float32)  # [768, C]
    return np.ascontiguousarray(wT.reshape(HT, 128, c_dim), dtype=np.float32)


def build_program():
    """Builds the SPMD Bass program (identical on all cores)."""
    nc = bacc.Bacc("TRN2", target_bir_lowering=False, debug=False,
                   num_devices=N_CORES)

    dram = {}

    def din(name, shape):
        dram[name] = nc.dram_tensor(name, list(shape), F32, kind="ExternalInput")

    def dout(name, shape):
        dram[name] = nc.dram_tensor(name, list(shape), F32, kind="ExternalOutput")

    din("embT", (128, HT, S))        # embT[p, j, s] = emb[b, s, 128j+p]
    din("negmask", (96, NM_SLOTS, S))  # row i -> [32*((i//32)%3) + i%32, i//96]
    din("indsel", (96, 32, 128))     # indsel[32q+c, k, p] = (c == k)
    din("identity", (128, 128))
    din("ctx_col", (128, HT))        # ctx_col[p, j] = emb[b, cls_s, 128j+p]
    din("ner_sz_static", (128, EQ))  # rows 0:32 sz^T, rest zero
    din("emd_sz_static", (128, MQ))
    din("rel_sz_static", (128, RQ))  # rows 0:64 szpair^T, rest zero
    din("cr_sz_static", (128, R2Q))
    for n in ["ner_b1", "emd_b1", "rel_b1", "cr_b1", "rel_b2", "cr_b2"]:
        din(n, (128, HT))
    din("head_biases", (128, 4))     # cols: ner(10), emd(2), rel(8), cr(1)
    din("ner_wctxT", (HT, HT, 128, 128))
    din("emd_wctxT", (HT, HT, 128, 128))
    din("ner_w1T", (7, HT, 128, 128))
    din("emd_w1T", (7, HT, 128, 128))
    din("rel_w1T", (19, HT, 128, 128))
    din("cr_w1T", (19, HT, 128, 128))
    din("rel_w2T", (HT, HT, 128, 128))
    din("cr_w2T", (HT, HT, 128, 128))
    din("ner_headT", (HT, 128, NER_C))
    din("emd_headT", (HT, 128, 2))
    din("rel_headT", (HT, 128, REL_C))
    din("cr_headT", (HT, 128, 1))

    dout("ner_out", (NER_C, EQ))
    dout("emd_out", (2, MQ))
    dout("rel_out", (REL_C, RQ))
    dout("cr_out", (1, R2Q))

    with tile.TileContext(nc) as tc:
        _kernel_body(tc, dram)

    nc.compile()
    return nc


def _kernel_body(tc, dram):
    nc = tc.nc
    from contextlib import ExitStack
    ctx = ExitStack()
    with ctx:
        const_pool = ctx.enter_context(tc.tile_pool(name="const", bufs=1))
        pools_pool = ctx.enter_context(tc.tile_pool(name="pools", bufs=1))
        mpsum_pool = ctx.enter_context(tc.tile_pool(name="mpsum", bufs=3, space="PSUM"))
        w_pool = ctx.enter_context(tc.tile_pool(name="wts", bufs=1))
        mm_psum = ctx.enter_context(tc.tile_pool(name="mmpsum", bufs=2, space="PSUM"))
        act_pool = ctx.enter_context(tc.tile_pool(name="acts", bufs=1))
        misc_pool = ctx.enter_context(tc.tile_pool(name="misc", bufs=1))

        def load(name, shape, pool=const_pool):
            t = pool.tile(list(shape), F32, name=name, tag=name)
            nc.sync.dma_start(out=t[...], in_=dram[name][...])
            return t

        embT = load("embT", (128, HT, S))
        negmask = load("negmask", (96, NM_SLOTS, S))
        indsel = load("indsel", (96, 32, 128))
        identity = load("identity", (128, 128))
        ctx_col = load("ctx_col", (128, HT))
        statics = {n: load(n, (128, q)) for n, q in
                   [("ner_sz_static", EQ), ("emd_sz_static", MQ),
                    ("rel_sz_static", RQ), ("cr_sz_static", R2Q)]}
        biases = {n: load(n, (128, HT)) for n in
                  ["ner_b1", "emd_b1", "rel_b1", "cr_b1", "rel_b2", "cr_b2"]}
        head_biases = load("head_biases", (128, 4))

        wrow_cache = {}

        def wrow(name, ct):
            """Loads dram[name][ct] = [HT, 128, 128] as one SBUF tile
            [128, HT, 128]; lhsT for pgroup pg is wrow[:, pg, :]."""
            key = (name, ct)
            if key not in wrow_cache:
                t = w_pool.tile([128, HT, 128], F32, name="wrow", tag="wrow", bufs=32)
                nc.sync.dma_start(out=t[...], in_=dram[name][ct].transpose([1, 0, 2]))
                wrow_cache[key] = t
            return wrow_cache[key]

        def wtile(name, idx, shape):
            if len(idx) == 2 and shape == (128, 128):
                return wrow(name, idx[0])[:, idx[1], :]
            t = w_pool.tile(list(shape), F32, name="w", tag="w", bufs=8)
            nc.sync.dma_start(out=t[...], in_=dram[name][idx])
            return t

        # ---- ctx projection: v = Wctx^T.T @ ctx; bias_vec = v + b1 ----
        bias_vec = {}
        for path, wname, bname in [("ner", "ner_wctxT", "ner_b1"),
                                   ("emd", "emd_wctxT", "emd_b1")]:
            bv = misc_pool.tile([128, HT], F32, name=f"bv_{path}", tag=f"bv_{path}")
            for pg in range(HT):
                ps = mm_psum.tile([128, 1], F32, name="vps", tag="mmps")
                for ct in range(HT):
                    w = wtile(wname, (ct, pg), (128, 128))
                    nc.tensor.matmul(ps[...], w[...], ctx_col[:, ct:ct + 1],
                                     start=(ct == 0), stop=(ct == HT - 1))
                nc.vector.tensor_add(bv[:, pg:pg + 1], ps[...],
                                     biases[bname][:, pg:pg + 1])
            bias_vec[path] = bv

        # ---- masked max pools ----
        ptiles = {
            "ent": pools_pool.tile([128, HT, EQ], F32, name="entp", tag="entp"),
            "men": pools_pool.tile([128, HT, MQ], F32, name="menp", tag="menp"),
            "rctx": pools_pool.tile([128, HT, RQ], F32, name="rctxp", tag="rctxp"),
            "cctx": pools_pool.tile([128, HT, R2Q], F32, name="cctxp", tag="cctxp"),
            "rp0": pools_pool.tile([128, HT, RQ], F32, name="rp0", tag="rp0"),
            "rp1": pools_pool.tile([128, HT, RQ], F32, name="rp1", tag="rp1"),
            "cp0": pools_pool.tile([128, HT, R2Q], F32, name="cp0", tag="cp0"),
            "cp1": pools_pool.tile([128, HT, R2Q], F32, name="cp1", tag="cp1"),
        }
        layout = [("ent", ROW_ENT, EQ), ("men", ROW_MEN, MQ),
                  ("rctx", ROW_RCTX, RQ), ("cctx", ROW_CCTX, R2Q),
                  ("rp0", ROW_RP0, RQ), ("rp1", ROW_RP1, RQ),
                  ("cp0", ROW_CP0, R2Q), ("cp1", ROW_CP1, R2Q)]

        NB = 2  # spans per fused psum/reduce block
        for key, row0, n in layout:
            ptile = ptiles[key]
            for blk in range(0, n, NB):
                for j in range(HT):
                    ps = mpsum_pool.tile([128, NB, S], F32, name="maskps",
                                         tag="maskps")
                    for u in range(NB):
                        row = row0 + blk + u
                        qd, k, slot = (row // 32) % 3, row % 32, row // 96
                        nc.tensor.matmul(ps[:, u, :], identity[...],
                                         embT[:, j, :], start=True, stop=False)
                        nc.tensor.matmul(ps[:, u, :],
                                         indsel[32 * qd:32 * qd + 32, k, :],
                                         negmask[32 * qd:32 * qd + 32, slot, :],
                                         start=False, stop=True)
                    nc.vector.tensor_reduce(ptile[:, j, blk:blk + NB], ps[...],
                                            axis=mybir.AxisListType.X,
                                            op=ALU.max)

        # ---- classifier MLPs ----
        def l1_span(path, pool_t, nspans):
            h1 = act_pool.tile([128, HT, nspans], F32, name=f"h1_{path}", tag=f"h1_{path}")
            for pg in range(HT):
                ps = mm_psum.tile([128, nspans], F32, name="l1ps", tag="mmps")
                for ct in range(7):
                    w = wtile(f"{path}_w1T", (ct, pg), (128, 128))
                    rhs = (pool_t[:, ct, :] if ct < HT
                           else statics[f"{path}_sz_static"][...])
                    nc.tensor.matmul(ps[...], w[...], rhs,
                                     start=(ct == 0), stop=(ct == 6))
                nc.scalar.activation(h1[:, pg, :], ps[...], AF.Relu,
                                     bias=bias_vec[path][:, pg:pg + 1])
            return h1

        def l1_rel(path, ctx_t, p0, p1, nspans):
            h1 = act_pool.tile([128, HT, nspans], F32, name=f"h1_{path}", tag=f"h1_{path}")
            for pg in range(HT):
                ps = mm_psum.tile([128, nspans], F32, name="l1ps", tag="mmps")
                for ct in range(19):
                    w = wtile(f"{path}_w1T", (ct, pg), (128, 128))
                    if ct < 6:
                        rhs = ctx_t[:, ct, :]
                    elif ct < 12:
                        rhs = p0[:, ct - 6, :]
                    elif ct < 18:
                        rhs = p1[:, ct - 12, :]
                    else:
                        rhs = statics[f"{path}_sz_static"][...]
                    nc.tensor.matmul(ps[...], w[...], rhs,
                                     start=(ct == 0), stop=(ct == 18))
                nc.scalar.activation(h1[:, pg, :], ps[...], AF.Relu,
                                     bias=biases[f"{path}_b1"][:, pg:pg + 1])
            return h1

        def l2(path, h1, nspans):
            h2 = act_pool.tile([128, HT, nspans], F32, name=f"h2_{path}", tag=f"h2_{path}")
            for pg in range(HT):
                ps = mm_psum.tile([128, nspans], F32, name="l2ps", tag="mmps")
                for ct in range(HT):
                    w = wtile(f"{path}_w2T", (ct, pg), (128, 128))
                    nc.tensor.matmul(ps[...], w[...], h1[:, ct, :],
                                     start=(ct == 0), stop=(ct == HT - 1))
                nc.scalar.activation(h2[:, pg, :], ps[...], AF.Relu,
                                     bias=biases[f"{path}_b2"][:, pg:pg + 1])
            return h2

        def head(path, hin, nspans, c_dim, bias_col, out_name):
            ps = mm_psum.tile([c_dim, nspans], F32, name="headps", tag="mmps")
            for ct in range(HT):
                w = wtile(f"{path}_headT", (ct,), (128, c_dim))
                nc.tensor.matmul(ps[...], w[...], hin[:, ct, :],
                                 start=(ct == 0), stop=(ct == HT - 1))
            out_sb = misc_pool.tile([c_dim, nspans], F32, name=f"out_{path}", tag=f"out_{path}")
            nc.scalar.activation(out_sb[...], ps[...], AF.Identity,
                                 bias=head_biases[0:c_dim, bias_col:bias_col + 1])
            nc.sync.dma_start(out=dram[out_name][...], in_=out_sb[...])

        head("ner", l1_span("ner", ptiles["ent"], EQ), EQ, NER_C, 0, "ner_out")
        head("emd", l1_span("emd", ptiles["men"], MQ), MQ, 2, 1, "emd_out")
        rel_h2 = l2("rel", l1_rel("rel", ptiles["rctx"], ptiles["rp0"],
                                  ptiles["rp1"], RQ), RQ)
        head("rel", rel_h2, RQ, REL_C, 2, "rel_out")
        cr_h2 = l2("cr", l1_rel("cr", ptiles["cctx"], ptiles["cp0"],
                                ptiles["cp1"], R2Q), R2Q)
        head("cr", cr_h2, R2Q, 1, 3, "cr_out")


def _pack_core_inputs(b, q, host):
    """Per-core input map (all f32, pre-packed partition-major)."""
    im = {}
    im["embT"] = host["embT"][b]
    im["indsel"] = host["indsel"]
    im["identity"] = np.eye(128, dtype=np.float32)
    im["ctx_col"] = host["ctx_col"][b]

    sl = lambda a, n: a[b][q * n:(q + 1) * n]          # quarter rows [n, S]
    rels, refs = host["rels"][b], host["refs"][b]      # [R, 2] ints
    myrels = rels[q * RQ:(q + 1) * RQ]
    myrefs = refs[q * R2Q:(q + 1) * R2Q]
    rows = np.concatenate([
        sl(host["neg_ent"], EQ),
        sl(host["neg_men"], MQ),
        sl(host["neg_rel"], RQ),
        sl(host["neg_ref"], R2Q),
        host["neg_ent"][b][myrels[:, 0]],
        host["neg_ent"][b][myrels[:, 1]],
        host["neg_men"][b][myrefs[:, 0]],
        host["neg_men"][b][myrefs[:, 1]],
    ], axis=0)
    nm = np.zeros((96, NM_SLOTS, S), np.float32)
    for i in range(NPOOL):
        nm[32 * ((i // 32) % 3) + i % 32, i // 96] = rows[i]
    im["negmask"] = np.ascontiguousarray(nm)

    def sz_static(sz_rows, n):
        t = np.zeros((128, n), np.float32)
        t[:sz_rows.shape[1], :] = sz_rows.T
        return t

    im["ner_sz_static"] = sz_static(sl(host["sz_ent"], EQ), EQ)
    im["emd_sz_static"] = sz_static(sl(host["sz_men"], MQ), MQ)
    im["rel_sz_static"] = sz_static(sl(host["szp_rel"], RQ), RQ)
    im["cr_sz_static"] = sz_static(sl(host["szp_ref"], R2Q), R2Q)

    for n in ["ner_b1", "emd_b1", "rel_b1", "cr_b1", "rel_b2", "cr_b2",
              "head_biases", "ner_wctxT", "emd_wctxT", "ner_w1T", "emd_w1T",
              "rel_w1T", "cr_w1T", "rel_w2T", "cr_w2T", "ner_headT",
              "emd_headT", "rel_headT", "cr_headT"]:
        im[n] = host[n]
    return im


_PROGRAM_CACHE = []


def kernel(**inputs):
    inputs = {k: np.asarray(v) for k, v in inputs.items()}
    emb = inputs["token_embedding"].astype(np.float32)

    def negm(mask):
        return np.where(mask != 0, 0.0, NEG).astype(np.float32)

    ind = np.zeros((96, 32, 128), np.float32)
    for qd in range(3):
        for k in range(32):
            ind[32 * qd + k, k, :] = 1.0
    host = {
        "indsel": ind,
        "rels": np.asarray(inputs["relations"]),
        "refs": np.asarray(inputs["references"]),
        "neg_ent": negm(inputs["entity_masks"]),
        "neg_men": negm(inputs["mention_masks"]),
        "neg_rel": negm(inputs["rel_masks"]),
        "neg_ref": negm(inputs["references_masks"]),
        "sz_ent": inputs["ner_size_emb"][inputs["entity_sizes"]].astype(np.float32),
        "sz_men": inputs["emd_size_emb"][inputs["mention_sizes"]].astype(np.float32),
    }
    host["szp_rel"] = host["sz_ent"][
        np.arange(B)[:, None, None], host["rels"]].reshape(B, R, 2 * SIZE_E)
    host["szp_ref"] = host["sz_men"][
        np.arange(B)[:, None, None], host["refs"]].reshape(B, R2, 2 * SIZE_E)

    host["embT"] = np.ascontiguousarray(
        emb.transpose(0, 2, 1).reshape(B, HT, 128, S).transpose(0, 2, 1, 3),
        dtype=np.float32)
    ctx_cols = []
    for b in range(B):
        cls_s = int(np.argmax(np.asarray(inputs["input_ids"][b]) == CLS))
        ctx_cols.append(np.ascontiguousarray(
            emb[b, cls_s].reshape(HT, 128).T, dtype=np.float32))
    host["ctx_col"] = np.stack(ctx_cols)

    ner_rep_w = inputs["ner_rep_w"].astype(np.float32)
    emd_rep_w = inputs["emd_rep_w"].astype(np.float32)
    host["ner_wctxT"] = _tiles_T(ner_rep_w[:, :H], H)
    host["emd_wctxT"] = _tiles_T(emd_rep_w[:, :H], H)
    host["ner_w1T"] = _tiles_T(ner_rep_w[:, H:], 7 * 128)
    host["emd_w1T"] = _tiles_T(emd_rep_w[:, H:], 7 * 128)
    host["rel_w1T"] = _tiles_T(inputs["rel_w1"].astype(np.float32), 19 * 128)
    host["cr_w1T"] = _tiles_T(inputs["cr_w1"].astype(np.float32), 19 * 128)
    host["rel_w2T"] = _tiles_T(inputs["rel_w2"].astype(np.float32), H)
    host["cr_w2T"] = _tiles_T(inputs["cr_w2"].astype(np.float32), H)
    host["ner_headT"] = _head_T(inputs["ner_head_w"].astype(np.float32), NER_C)
    host["emd_headT"] = _head_T(inputs["emd_head_w"].astype(np.float32), 2)
    host["rel_headT"] = _head_T(inputs["rel_w3"].astype(np.float32), REL_C)
    host["cr_headT"] = _head_T(inputs["cr_w3"].astype(np.float32), 1)

    def pgmaj(v):
        return np.ascontiguousarray(
            np.asarray(v, np.float32).reshape(HT, 128).T, dtype=np.float32)

    for n, src in [("ner_b1", "ner_rep_b"), ("emd_b1", "emd_rep_b"),
                   ("rel_b1", "rel_b1"), ("cr_b1", "cr_b1"),
                   ("rel_b2", "rel_b2"), ("cr_b2", "cr_b2")]:
        host[n] = pgmaj(inputs[src])
    hb = np.zeros((128, 4), np.float32)
    hb[:NER_C, 0] = inputs["ner_head_b"]
    hb[:2, 1] = inputs["emd_head_b"]
    hb[:REL_C, 2] = inputs["rel_b3"]
    hb[:1, 3] = inputs["cr_b3"]
    host["head_biases"] = hb

    in_maps = [_pack_core_inputs(c // GROUP, c % GROUP, host)
               for c in range(N_CORES)]

    if not _PROGRAM_CACHE:
        _PROGRAM_CACHE.append(build_program())
    nc = _PROGRAM_CACHE[0]

    res = run_bass_kernel_spmd(nc, in_maps, list(range(N_CORES)))
    return assemble_outputs(res.results)


def assemble_outputs(results):
    entity_clf = np.zeros((B, E, NER_C), np.float32)
    mention_clf = np.zeros((B, M, 2), np.float32)
    rel_clf = np.zeros((B, R, REL_C), np.float32)
    ref_clf = np.zeros((B, R2, 1), np.float32)
    for c in range(N_CORES):
        b, q = c // GROUP, c % GROUP
        entity_clf[b, q * EQ:(q + 1) * EQ] = results[c]["ner_out"].T
        mention_clf[b, q * MQ:(q + 1) * MQ] = results[c]["emd_out"].T
        rel_clf[b, q * RQ:(q + 1) * RQ] = results[c]["rel_out"].T
        ref_clf[b, q * R2Q:(q + 1) * R2Q] = results[c]["cr_out"].T
    return entity_clf, rel_clf, mention_clf, ref_clf
